# revision 31
# baseline (speedup 1.0000x reference)
"""Trainium2 Bass kernel for nn_DigitConvolutionalModel.

Model: out = relu(conv2d_valid(x.reshape(28,28), conv_w).reshape(676) @ w1 + b1) @ w2 + b2

Strategy:
  - The 3x3 valid conv is a linear map C [784, 676]; fold it into the first
    FC layer on the host: W1' = C @ w1  [784, 300]. The device then runs a
    plain 2-layer MLP: out = relu(x @ W1' + b1) @ w2 + b2.
  - Pure data parallel over 8 NeuronCores: batch 65536 -> 8192 per core.
  - Feature-major device layout: host supplies x.T per core so the
    contraction dim sits on SBUF partitions for both matmul operands.
    Layer 1 computes g = relu(W1'.T @ x.T + b1) as [300, batch] (features on
    partitions -> b1 is a per-partition ACT bias). Layer 2 reuses g directly
    as the moving operand: out.T = w2.T @ g + b2 [10, batch]. Host transposes
    the tiny [10, 65536] result back.
  - All feature dims zero-padded to multiples of 128 (784->896, 300->384) so
    every tile is a uniform [128, *]: single 3D-AP DMAs, no edge cases, and
    the zero-padding contributes exactly 0 through matmul/relu.
  - bf16 matmul inputs (1 PE cycle/row vs 4 for fp32), fp32 PSUM accumulate.
  - Batch tiles processed in pairs sharing the stationary weight per (k, m)
    chunk; layer 2 runs software-pipelined one pair behind layer 1 so the
    TensorEngine never waits on the ACT relu that produces g.
"""

import numpy as np
import ml_dtypes

_B = 65536
_NCORES = 8
_BSH = _B // _NCORES  # 8192 batch rows per core
_N = 512  # batch columns per matmul (one fp32 PSUM bank)
_KP = 896  # padded input features (784 -> 7 chunks of 128)
_MP = 384  # padded hidden features (300 -> 3 chunks of 128)
_NK = _KP // 128  # 7
_NM = _MP // 128  # 3
_NPAIR = _BSH // (2 * _N)  # 8 pairs of 512-col batch tiles

_state = {}


def _build_nc():
    import concourse.tile as tile
    from concourse import bacc, mybir
    from contextlib import ExitStack

    dt = mybir.dt
    AF = mybir.ActivationFunctionType

    nc = bacc.Bacc(
        "TRN2",
        target_bir_lowering=False,
        debug=False,
        enable_asserts=False,
        num_devices=_NCORES,
    )

    xt = nc.dram_tensor("xt", [_KP, _BSH], dt.bfloat16, kind="ExternalInput").ap()
    w1 = nc.dram_tensor("w1", [_KP, _MP], dt.bfloat16, kind="ExternalInput").ap()
    b1 = nc.dram_tensor("b1", [_MP, 1], dt.float32, kind="ExternalInput").ap()
    w2 = nc.dram_tensor("w2", [_MP, 10], dt.bfloat16, kind="ExternalInput").ap()
    b2 = nc.dram_tensor("b2", [10, 1], dt.float32, kind="ExternalInput").ap()
    outT = nc.dram_tensor("outT", [10, _BSH], dt.float32, kind="ExternalOutput").ap()

    # Partition-chunked DRAM views: [(chunk, p), cols] -> [p, chunk, cols]
    xt_r = xt.rearrange("(k p) c -> p k c", p=128)  # [128, 7, 8192]
    w1_r = w1.rearrange("(k p) m -> p k m", p=128)  # [128, 7, 384]
    b1_r = b1.rearrange("(m p) one -> p m one", p=128)  # [128, 3, 1]
    w2_r = w2.rearrange("(m p) o -> p m o", p=128)  # [128, 3, 10]

    with tile.TileContext(nc) as tc, ExitStack() as ctx:
        wpool = ctx.enter_context(tc.tile_pool(name="wpool", bufs=1))
        xpool = ctx.enter_context(tc.tile_pool(name="xpool", bufs=3))
        gpool = ctx.enter_context(tc.tile_pool(name="gpool", bufs=2))
        ppool = ctx.enter_context(tc.tile_pool(name="ppool", bufs=5, space="PSUM"))
        pm2pool = ctx.enter_context(tc.tile_pool(name="pm2pool", bufs=1, space="PSUM"))
        p2pool = ctx.enter_context(tc.tile_pool(name="p2pool", bufs=2, space="PSUM"))
        opool = ctx.enter_context(tc.tile_pool(name="opool", bufs=2))

        # PE warm-up: ~45 dependency-free matmuls on a zeroed scratch tile keep
        # the TensorEngine busy through the HAM activity window while the first
        # real DMAs land, so the real matmul stream starts at 2.4 GHz.
        warm_in = wpool.tile([128, 128], dt.bfloat16, name="warm_in", tag="warm_in")
        nc.gpsimd.memset(warm_in[:], 0.0)
        warm_ps = p2pool.tile([128, 128], dt.float32, name="warm_ps", tag="ps2")
        for _ in range(14):
            nc.tensor.matmul(
                out=warm_ps[:], lhsT=warm_in[:], rhs=warm_in[:], start=True, stop=True
            )

        # Stationary weights + first batch pair, interleaved per k-chunk so the
        # first matmul only waits on its own (w1[k0], xt[k0]) slices instead of
        # the whole 2.5 MB prologue transfer. All 7 k-chunks are full 128-row
        # (zero-padded) so every matmul stays in the uniform full-array mode —
        # row-group mode switches cost ~2x the padding they save.
        w1sb = wpool.tile([128, _NK, _MP], dt.bfloat16, name="w1sb", tag="w1sb")
        xt0 = xpool.tile([128, _NK, 2 * _N], dt.bfloat16, name="xt_0", tag="xt")
        for ki in range(_NK):
            nc.sync.dma_start(out=w1sb[:, ki, :], in_=w1_r[:, ki, :])
            nc.sync.dma_start(out=xt0[:, ki, :], in_=xt_r[:, ki, 0 : 2 * _N])
        # Prefetch the second pair before the small bias/w2 loads so its 1.8 MB
        # transfer completes before pair 0's compute finishes.
        xt1 = xpool.tile([128, _NK, 2 * _N], dt.bfloat16, name="xt_1", tag="xt")
        nc.sync.dma_start(out=xt1[:], in_=xt_r[:, :, 2 * _N : 4 * _N])
        b1sb = wpool.tile([128, _NM, 1], dt.float32, name="b1sb", tag="b1sb")
        nc.sync.dma_start(out=b1sb[:], in_=b1_r[:])
        w2sb = wpool.tile([128, _NM, 10], dt.bfloat16, name="w2sb", tag="w2sb")
        nc.sync.dma_start(out=w2sb[:], in_=w2_r[:])
        b2sb = wpool.tile([10, 1], dt.float32, name="b2sb", tag="b2sb")
        nc.sync.dma_start(out=b2sb[:], in_=b2[:, :])
        # The m2 chunk (44 real rows of 300) is col-tiled: batch half j=1 lands
        # at psum/sbuf partitions 64.. so its bias, weights, and layer-2 operand
        # need partition-64-aligned replicas.
        _M2 = 300 - 256  # 44
        b1rep = wpool.tile([128, 1], dt.float32, name="b1rep", tag="b1rep")
        nc.sync.dma_start(out=b1rep[64 : 64 + _M2, :], in_=b1_r[0:_M2, 2, :])
        w2rep = wpool.tile([128, 10], dt.bfloat16, name="w2rep", tag="w2rep")
        nc.gpsimd.memset(w2rep[:], 0.0)
        nc.sync.dma_start(out=w2rep[64 : 64 + _M2, :], in_=w2_r[0:_M2, 2, :])

        def layer2(prev_g, prev_c0):
            """Second layer + output store for the pair at column prev_c0."""
            for j in range(2):
                ps2 = p2pool.tile([10, _N], dt.float32, name=f"ps2_{prev_c0}_{j}", tag="ps2")
                for mi in range(2):
                    nc.tensor.matmul(
                        out=ps2[:],
                        lhsT=w2sb[:, mi, :],
                        rhs=prev_g[(mi, j)][:],
                        start=(mi == 0),
                        stop=False,
                    )
                # Full 128-row matmul: the weight rows outside the real 44-row
                # m2 chunk are zero, and the matching g rows are memset to 0.
                nc.tensor.matmul(
                    out=ps2[:],
                    lhsT=(w2sb[:, 2, :] if j == 0 else w2rep[:]),
                    rhs=prev_g[(2, j)][:],
                    start=False,
                    stop=True,
                )
                store_out(ps2, prev_c0, j)

        def store_out(ps2, base_c0, j):
            ob = opool.tile([10, _N], dt.float32, name=f"ob_{base_c0}_{j}", tag="ob")
            if j == 0:
                nc.scalar.activation(ob[:], ps2[:], AF.Identity, bias=b2sb[:], scale=1.0)
            else:
                nc.vector.tensor_scalar(
                    ob[:], ps2[:], b2sb[:], None, mybir.AluOpType.add
                )
            c0 = base_c0 + j * _N
            nc.sync.dma_start(out=outT[:, c0 : c0 + _N], in_=ob[:])

        prev_g = None
        prev_c0 = 0
        for pair in range(_NPAIR):
            last = pair == _NPAIR - 1
            c0 = pair * 2 * _N
            if pair == 0:
                xtile = xt0
            elif pair == 1:
                xtile = xt1
            else:
                # Both 512-col batch tiles of the pair in one 1.8 MB DMA.
                xtile = xpool.tile(
                    [128, _NK, 2 * _N], dt.bfloat16, name=f"xt_{pair}", tag="xt"
                )
                nc.sync.dma_start(out=xtile[:], in_=xt_r[:, :, c0 : c0 + 2 * _N])

            cur_g = {}
            for mi in range(2):
                ps = [
                    ppool.tile([128, _N], dt.float32, name=f"ps_{pair}_{mi}_{j}", tag="ps")
                    for j in range(2)
                ]
                for ki in range(_NK):
                    for j in range(2):
                        nc.tensor.matmul(
                            out=ps[j][:],
                            lhsT=w1sb[:, ki, mi * 128 : (mi + 1) * 128],
                            rhs=xtile[:, ki, j * _N : (j + 1) * _N],
                            start=(ki == 0),
                            stop=(ki == _NK - 1),
                        )
                for j in range(2):
                    g = gpool.tile(
                        [128, _N], dt.bfloat16, name=f"g_{pair}_{mi}_{j}", tag=f"g{mi}{j}"
                    )
                    if j == 0:
                        # Split the relus across ACT and DVE so neither engine
                        # serializes the psum drain.
                        nc.scalar.activation(
                            g[:], ps[j][:], AF.Relu, bias=b1sb[:, mi, :], scale=1.0
                        )
                    else:
                        nc.vector.tensor_scalar(
                            g[:], ps[j][:], b1sb[:, mi, :], 0.0,
                            mybir.AluOpType.add, mybir.AluOpType.max,
                        )
                    cur_g[(mi, j)] = g
                if mi == 0 and prev_g is not None:
                    # Software-pipelined layer 2 for the previous pair: by now
                    # ACT has had a full m-block of matmul time to finish its g.
                    layer2(prev_g, prev_c0)
                if last and mi == 1:
                    # Final pair: start its layer 2 in-stream (m0/m1 terms) so
                    # only the m2 term remains on the tail critical path.
                    ps2L = {}
                    for j in range(2):
                        ps2L[j] = p2pool.tile(
                            [10, _N], dt.float32, name=f"ps2_last_{j}", tag="ps2"
                        )
                        for mm in range(2):
                            nc.tensor.matmul(
                                out=ps2L[j][:],
                                lhsT=w2sb[:, mm, :],
                                rhs=cur_g[(mm, j)][:],
                                start=(mm == 0),
                                stop=False,
                            )

            # m2 chunk (44 output rows): both batch halves run concurrently as
            # col-tiled matmuls — j=0 writes psum partitions 0..43 (col group
            # 0), j=1 writes partitions 64..107 (col group 64) of one bank.
            psm2 = pm2pool.tile([128, _N], dt.float32, name=f"psm2_{pair}", tag="psm2")
            for ki in range(_NK):
                for j in range(2):
                    nc.tensor.matmul(
                        out=psm2[64 * j : 64 * j + _M2, :],
                        lhsT=w1sb[:, ki, 256 : 256 + _M2],
                        rhs=xtile[:, ki, j * _N : (j + 1) * _N],
                        start=(ki == 0),
                        stop=(ki == _NK - 1),
                        tile_position=(0, 64 * j),
                    )
            # g tiles are full 128 rows with the unused rows zeroed so layer 2
            # can use uniform full-row matmuls (0-weight x 0-value, never NaN).
            g20 = gpool.tile([128, _N], dt.bfloat16, name=f"g_{pair}_2_0", tag="g20")
            nc.gpsimd.memset(g20[32:64, :], 0.0)  # 32-aligned; relu rewrites 32..43
            nc.gpsimd.memset(g20[64:128, :], 0.0)
            nc.scalar.activation(
                g20[0:_M2, :], psm2[0:_M2, :], AF.Relu, bias=b1sb[0:_M2, 2, :], scale=1.0
            )
            g21 = gpool.tile([128, _N], dt.bfloat16, name=f"g_{pair}_2_1", tag="g21")
            nc.gpsimd.memset(g21[0:64, :], 0.0)
            nc.gpsimd.memset(g21[96:128, :], 0.0)  # 32-aligned; relu rewrites 96..107
            nc.vector.tensor_scalar(
                g21[64 : 64 + _M2, :], psm2[64 : 64 + _M2, :], b1rep[64 : 64 + _M2, :],
                0.0, mybir.AluOpType.add, mybir.AluOpType.max,
            )
            cur_g[(2, 0)] = g20
            cur_g[(2, 1)] = g21
            if last:
                for j in range(2):
                    nc.tensor.matmul(
                        out=ps2L[j][:],
                        lhsT=(w2sb[:, 2, :] if j == 0 else w2rep[:]),
                        rhs=cur_g[(2, j)][:],
                        start=False,
                        stop=True,
                    )
                    store_out(ps2L[j], c0, j)
            prev_g = cur_g
            prev_c0 = c0

    nc.compile()
    return nc


def _fold_conv(conv_w, w1):
    """W1' = C @ w1 where C [784, 676] is the linear map of the 3x3 valid conv."""
    C = np.zeros((784, 676), np.float64)
    cw = np.asarray(conv_w, np.float64)
    for di in range(3):
        for dj in range(3):
            for i in range(26):
                rows = (i + di) * 28 + dj + np.arange(26)
                C[rows, i * 26 + np.arange(26)] += cw[di, dj]
    return C @ np.asarray(w1, np.float64)  # [784, 300]


def _exec(inputs, trace=False, **run_kwargs):
    from concourse.bass_utils import run_bass_kernel_spmd

    x = np.asarray(inputs["x"], np.float32)
    bf16 = ml_dtypes.bfloat16

    w1f = np.zeros((_KP, _MP), bf16)
    w1f[:784, :300] = _fold_conv(inputs["conv_w"], inputs["w1"]).astype(bf16)
    b1c = np.zeros((_MP, 1), np.float32)
    b1c[:300, 0] = np.asarray(inputs["b1"], np.float32)
    w2b = np.zeros((_MP, 10), bf16)
    w2b[:300] = np.asarray(inputs["w2"], np.float32).astype(bf16)
    b2c = np.ascontiguousarray(np.asarray(inputs["b2"], np.float32).reshape(10, 1))

    if "nc" not in _state:
        _state["nc"] = _build_nc()
    nc = _state["nc"]

    xb = x.astype(bf16)  # [65536, 784]
    in_maps = []
    for c in range(_NCORES):
        sh = np.zeros((_KP, _BSH), bf16)
        sh[:784] = xb[c * _BSH : (c + 1) * _BSH, :].T  # [784, 8192]
        in_maps.append({"xt": sh, "w1": w1f, "b1": b1c, "w2": w2b, "b2": b2c})

    res = run_bass_kernel_spmd(
        nc, in_maps, list(range(_NCORES)), trace=trace, **run_kwargs
    )
    outs = [res.results[c]["outT"] for c in range(_NCORES)]  # each [10, 8192]
    out = np.concatenate(outs, axis=1).T  # [65536, 10]
    return np.ascontiguousarray(out, dtype=np.float32), res


def kernel(**inputs):
    out, _ = _exec(inputs, trace=False)
    return out


# revision 35
# speedup vs baseline: 1.0244x; 1.0244x over previous
"""Trainium2 Bass kernel for nn_DigitConvolutionalModel.

Model: out = relu(conv2d_valid(x.reshape(28,28), conv_w).reshape(676) @ w1 + b1) @ w2 + b2

Strategy:
  - The 3x3 valid conv is a linear map C [784, 676]; fold it into the first
    FC layer on the host: W1' = C @ w1  [784, 300]. The device then runs a
    plain 2-layer MLP: out = relu(x @ W1' + b1) @ w2 + b2.
  - Pure data parallel over 8 NeuronCores: batch 65536 -> 8192 per core.
  - Feature-major device layout: host supplies x.T per core so the
    contraction dim sits on SBUF partitions for both matmul operands.
    Layer 1 computes g = relu(W1'.T @ x.T + b1) as [300, batch] (features on
    partitions -> b1 is a per-partition ACT bias). Layer 2 reuses g directly
    as the moving operand: out.T = w2.T @ g + b2 [10, batch]. Host transposes
    the tiny [10, 65536] result back.
  - All feature dims zero-padded to multiples of 128 (784->896, 300->384) so
    every tile is a uniform [128, *]: single 3D-AP DMAs, no edge cases, and
    the zero-padding contributes exactly 0 through matmul/relu.
  - bf16 matmul inputs (1 PE cycle/row vs 4 for fp32), fp32 PSUM accumulate.
  - Batch tiles processed in pairs sharing the stationary weight per (k, m)
    chunk; layer 2 runs software-pipelined one pair behind layer 1 so the
    TensorEngine never waits on the ACT relu that produces g.
"""

import numpy as np
import ml_dtypes

_B = 65536
_NCORES = 8
_BSH = _B // _NCORES  # 8192 batch rows per core
_N = 512  # batch columns per matmul (one fp32 PSUM bank)
_KP = 896  # padded input features (784 -> 7 chunks of 128)
_MP = 384  # padded hidden features (300 -> 3 chunks of 128)
_NK = _KP // 128  # 7
_NM = _MP // 128  # 3
_NPAIR = _BSH // (2 * _N)  # 8 pairs of 512-col batch tiles

_state = {}


def _build_nc():
    import concourse.tile as tile
    from concourse import bacc, mybir
    from contextlib import ExitStack

    dt = mybir.dt
    AF = mybir.ActivationFunctionType

    nc = bacc.Bacc(
        "TRN2",
        target_bir_lowering=False,
        debug=False,
        enable_asserts=False,
        num_devices=_NCORES,
    )

    xt = nc.dram_tensor("xt", [_KP, _BSH], dt.bfloat16, kind="ExternalInput").ap()
    w1 = nc.dram_tensor("w1", [_KP, _MP], dt.bfloat16, kind="ExternalInput").ap()
    b1 = nc.dram_tensor("b1", [_MP, 1], dt.float32, kind="ExternalInput").ap()
    w2 = nc.dram_tensor("w2", [_MP, 10], dt.bfloat16, kind="ExternalInput").ap()
    b2 = nc.dram_tensor("b2", [10, 1], dt.float32, kind="ExternalInput").ap()
    outT = nc.dram_tensor("outT", [10, _BSH], dt.float32, kind="ExternalOutput").ap()

    # Partition-chunked DRAM views: [(chunk, p), cols] -> [p, chunk, cols]
    xt_r = xt.rearrange("(k p) c -> p k c", p=128)  # [128, 7, 8192]
    w1_r = w1.rearrange("(k p) m -> p k m", p=128)  # [128, 7, 384]
    b1_r = b1.rearrange("(m p) one -> p m one", p=128)  # [128, 3, 1]
    w2_r = w2.rearrange("(m p) o -> p m o", p=128)  # [128, 3, 10]

    with tile.TileContext(nc) as tc, ExitStack() as ctx:
        wpool = ctx.enter_context(tc.tile_pool(name="wpool", bufs=1))
        xpool = ctx.enter_context(tc.tile_pool(name="xpool", bufs=3))
        gpool = ctx.enter_context(tc.tile_pool(name="gpool", bufs=2))
        ppool = ctx.enter_context(tc.tile_pool(name="ppool", bufs=5, space="PSUM"))
        pm2pool = ctx.enter_context(tc.tile_pool(name="pm2pool", bufs=1, space="PSUM"))
        p2pool = ctx.enter_context(tc.tile_pool(name="p2pool", bufs=2, space="PSUM"))
        opool = ctx.enter_context(tc.tile_pool(name="opool", bufs=2))

        # PE warm-up: ~45 dependency-free matmuls on a zeroed scratch tile keep
        # the TensorEngine busy through the HAM activity window while the first
        # real DMAs land, so the real matmul stream starts at 2.4 GHz.
        warm_in = wpool.tile([128, 128], dt.bfloat16, name="warm_in", tag="warm_in")
        nc.gpsimd.memset(warm_in[:], 0.0)
        warm_ps = p2pool.tile([128, 128], dt.float32, name="warm_ps", tag="ps2")
        for _ in range(14):
            nc.tensor.matmul(
                out=warm_ps[:], lhsT=warm_in[:], rhs=warm_in[:], start=True, stop=True
            )

        # Stationary weights + first batch pair, interleaved per k-chunk so the
        # first matmul only waits on its own (w1[k0], xt[k0]) slices instead of
        # the whole 2.5 MB prologue transfer. All 7 k-chunks are full 128-row
        # (zero-padded) so every matmul stays in the uniform full-array mode —
        # row-group mode switches cost ~2x the padding they save.
        w1sb = wpool.tile([128, _NK, _MP], dt.bfloat16, name="w1sb", tag="w1sb")
        xt0 = xpool.tile([128, _NK, 2 * _N], dt.bfloat16, name="xt_0", tag="xt")
        for ki in range(_NK):
            nc.sync.dma_start(out=w1sb[:, ki, :], in_=w1_r[:, ki, :])
            nc.sync.dma_start(out=xt0[:, ki, :], in_=xt_r[:, ki, 0 : 2 * _N])
        b1sb = wpool.tile([128, _NM, 1], dt.float32, name="b1sb", tag="b1sb")
        nc.sync.dma_start(out=b1sb[:], in_=b1_r[:])
        w2sb = wpool.tile([128, _NM, 10], dt.bfloat16, name="w2sb", tag="w2sb")
        nc.sync.dma_start(out=w2sb[:], in_=w2_r[:])
        b2sb = wpool.tile([10, 1], dt.float32, name="b2sb", tag="b2sb")
        nc.sync.dma_start(out=b2sb[:], in_=b2[:, :])
        # The m2 chunk (44 real rows of 300) is col-tiled: batch half j=1 lands
        # at psum/sbuf partitions 64.. so its bias, weights, and layer-2 operand
        # need partition-64-aligned replicas.
        _M2 = 300 - 256  # 44
        b1rep = wpool.tile([128, 1], dt.float32, name="b1rep", tag="b1rep")
        nc.sync.dma_start(out=b1rep[64 : 64 + _M2, :], in_=b1_r[0:_M2, 2, :])
        w2rep = wpool.tile([128, 10], dt.bfloat16, name="w2rep", tag="w2rep")
        nc.gpsimd.memset(w2rep[:], 0.0)
        nc.sync.dma_start(out=w2rep[64 : 64 + _M2, :], in_=w2_r[0:_M2, 2, :])

        def layer2(prev_g, prev_c0):
            """Second layer + output store for the pair at column prev_c0."""
            for j in range(2):
                ps2 = p2pool.tile([10, _N], dt.float32, name=f"ps2_{prev_c0}_{j}", tag="ps2")
                for mi in range(2):
                    nc.tensor.matmul(
                        out=ps2[:],
                        lhsT=w2sb[:, mi, :],
                        rhs=prev_g[(mi, j)][:],
                        start=(mi == 0),
                        stop=False,
                    )
                # Full 128-row matmul: the weight rows outside the real 44-row
                # m2 chunk are zero, and the matching g rows are memset to 0.
                nc.tensor.matmul(
                    out=ps2[:],
                    lhsT=(w2sb[:, 2, :] if j == 0 else w2rep[:]),
                    rhs=prev_g[(2, j)][:],
                    start=False,
                    stop=True,
                )
                store_out(ps2, prev_c0, j)

        def store_out(ps2, base_c0, j):
            ob = opool.tile([10, _N], dt.float32, name=f"ob_{base_c0}_{j}", tag="ob")
            if j == 0:
                nc.scalar.activation(ob[:], ps2[:], AF.Identity, bias=b2sb[:], scale=1.0)
            else:
                nc.vector.tensor_scalar(
                    ob[:], ps2[:], b2sb[:], None, mybir.AluOpType.add
                )
            c0 = base_c0 + j * _N
            nc.sync.dma_start(out=outT[:, c0 : c0 + _N], in_=ob[:])

        prev_g = None
        prev_c0 = 0
        for pair in range(_NPAIR):
            last = pair == _NPAIR - 1
            c0 = pair * 2 * _N
            if pair == 0:
                xtile = xt0
            else:
                # Both 512-col batch tiles of the pair in one 1.8 MB DMA.
                xtile = xpool.tile(
                    [128, _NK, 2 * _N], dt.bfloat16, name=f"xt_{pair}", tag="xt"
                )
                nc.sync.dma_start(out=xtile[:], in_=xt_r[:, :, c0 : c0 + 2 * _N])

            cur_g = {}
            for mi in range(2):
                ps = [
                    ppool.tile([128, _N], dt.float32, name=f"ps_{pair}_{mi}_{j}", tag="ps")
                    for j in range(2)
                ]
                for ki in range(_NK):
                    for j in range(2):
                        nc.tensor.matmul(
                            out=ps[j][:],
                            lhsT=w1sb[:, ki, mi * 128 : (mi + 1) * 128],
                            rhs=xtile[:, ki, j * _N : (j + 1) * _N],
                            start=(ki == 0),
                            stop=(ki == _NK - 1),
                        )
                for j in range(2):
                    g = gpool.tile(
                        [128, _N], dt.bfloat16, name=f"g_{pair}_{mi}_{j}", tag=f"g{mi}{j}"
                    )
                    if j == 0:
                        # Split the relus across ACT and DVE so neither engine
                        # serializes the psum drain.
                        nc.scalar.activation(
                            g[:], ps[j][:], AF.Relu, bias=b1sb[:, mi, :], scale=1.0
                        )
                    else:
                        nc.vector.tensor_scalar(
                            g[:], ps[j][:], b1sb[:, mi, :], 0.0,
                            mybir.AluOpType.add, mybir.AluOpType.max,
                        )
                    cur_g[(mi, j)] = g
                if mi == 0 and prev_g is not None:
                    # Software-pipelined layer 2 for the previous pair: by now
                    # ACT has had a full m-block of matmul time to finish its g.
                    layer2(prev_g, prev_c0)


            # m2 chunk (44 output rows): both batch halves run concurrently as
            # col-tiled matmuls — j=0 writes psum partitions 0..43 (col group
            # 0), j=1 writes partitions 64..107 (col group 64) of one bank.
            psm2 = pm2pool.tile([128, _N], dt.float32, name=f"psm2_{pair}", tag="psm2")
            for ki in range(_NK):
                for j in range(2):
                    nc.tensor.matmul(
                        out=psm2[64 * j : 64 * j + _M2, :],
                        lhsT=w1sb[:, ki, 256 : 256 + _M2],
                        rhs=xtile[:, ki, j * _N : (j + 1) * _N],
                        start=(ki == 0),
                        stop=(ki == _NK - 1),
                        tile_position=(0, 64 * j),
                    )
            if last:
                # Final pair: start its layer 2 in-stream (m0/m1 terms, whose g
                # tiles finished during the m2 block) so only the m2 term
                # remains on the tail critical path.
                ps2L = {}
                for j in range(2):
                    ps2L[j] = p2pool.tile(
                        [10, _N], dt.float32, name=f"ps2_last_{j}", tag="ps2"
                    )
                    for mm in range(2):
                        nc.tensor.matmul(
                            out=ps2L[j][:],
                            lhsT=w2sb[:, mm, :],
                            rhs=cur_g[(mm, j)][:],
                            start=(mm == 0),
                            stop=False,
                        )

            # g tiles are full 128 rows with the unused rows zeroed so layer 2
            # can use uniform full-row matmuls (0-weight x 0-value, never NaN).
            g20 = gpool.tile([128, _N], dt.bfloat16, name=f"g_{pair}_2_0", tag="g20")
            nc.gpsimd.memset(g20[32:64, :], 0.0)  # 32-aligned; relu rewrites 32..43
            nc.gpsimd.memset(g20[64:128, :], 0.0)
            nc.scalar.activation(
                g20[0:_M2, :], psm2[0:_M2, :], AF.Relu, bias=b1sb[0:_M2, 2, :], scale=1.0
            )
            g21 = gpool.tile([128, _N], dt.bfloat16, name=f"g_{pair}_2_1", tag="g21")
            nc.gpsimd.memset(g21[0:64, :], 0.0)
            nc.gpsimd.memset(g21[96:128, :], 0.0)  # 32-aligned; relu rewrites 96..107
            nc.vector.tensor_scalar(
                g21[64 : 64 + _M2, :], psm2[64 : 64 + _M2, :], b1rep[64 : 64 + _M2, :],
                0.0, mybir.AluOpType.add, mybir.AluOpType.max,
            )
            cur_g[(2, 0)] = g20
            cur_g[(2, 1)] = g21
            if last:
                for j in range(2):
                    nc.tensor.matmul(
                        out=ps2L[j][:],
                        lhsT=(w2sb[:, 2, :] if j == 0 else w2rep[:]),
                        rhs=cur_g[(2, j)][:],
                        start=False,
                        stop=True,
                    )
                    store_out(ps2L[j], c0, j)
            prev_g = cur_g
            prev_c0 = c0

    nc.compile()
    return nc


def _fold_conv(conv_w, w1):
    """W1' = C @ w1 where C [784, 676] is the linear map of the 3x3 valid conv."""
    C = np.zeros((784, 676), np.float64)
    cw = np.asarray(conv_w, np.float64)
    for di in range(3):
        for dj in range(3):
            for i in range(26):
                rows = (i + di) * 28 + dj + np.arange(26)
                C[rows, i * 26 + np.arange(26)] += cw[di, dj]
    return C @ np.asarray(w1, np.float64)  # [784, 300]


def _exec(inputs, trace=False, **run_kwargs):
    from concourse.bass_utils import run_bass_kernel_spmd

    x = np.asarray(inputs["x"], np.float32)
    bf16 = ml_dtypes.bfloat16

    w1f = np.zeros((_KP, _MP), bf16)
    w1f[:784, :300] = _fold_conv(inputs["conv_w"], inputs["w1"]).astype(bf16)
    b1c = np.zeros((_MP, 1), np.float32)
    b1c[:300, 0] = np.asarray(inputs["b1"], np.float32)
    w2b = np.zeros((_MP, 10), bf16)
    w2b[:300] = np.asarray(inputs["w2"], np.float32).astype(bf16)
    b2c = np.ascontiguousarray(np.asarray(inputs["b2"], np.float32).reshape(10, 1))

    if "nc" not in _state:
        _state["nc"] = _build_nc()
    nc = _state["nc"]

    xb = x.astype(bf16)  # [65536, 784]
    in_maps = []
    for c in range(_NCORES):
        sh = np.zeros((_KP, _BSH), bf16)
        sh[:784] = xb[c * _BSH : (c + 1) * _BSH, :].T  # [784, 8192]
        in_maps.append({"xt": sh, "w1": w1f, "b1": b1c, "w2": w2b, "b2": b2c})

    res = run_bass_kernel_spmd(
        nc, in_maps, list(range(_NCORES)), trace=trace, **run_kwargs
    )
    outs = [res.results[c]["outT"] for c in range(_NCORES)]  # each [10, 8192]
    out = np.concatenate(outs, axis=1).T  # [65536, 10]
    return np.ascontiguousarray(out, dtype=np.float32), res


def kernel(**inputs):
    out, _ = _exec(inputs, trace=False)
    return out
